# revision 1
# baseline (speedup 1.0000x reference)
"""Trainium2 Bass kernel for nn_EnsembleFormer (vq_codebook).

Strategy
--------
Every projected stream (p, k0, v0, k1, v1) in the reference is consumed only
AFTER spatial pooling (64x64 -> 8x8 agents, and 8x8 -> 2x2 clusters), and the
final output is a bilinear upsample of an 8x8 map followed by a 1x1 conv.
Pointwise (1x1) projections commute with average pooling and with bilinear
resize, so the whole network collapses to:

  pool x (64x64 -> 8x8)  ->  tiny GEMMs + clustering on the 8x8 grid
  ->  1x1 proj2 on the 8x8 grid  ->  bilinear upsample 8->64 (a matmul)

which turns a ~29 GFLOP problem into a memory-bound one (read x, write out).

Sharding: data-parallel over batch B=16 across 8 cores (2 batches/core),
weights replicated. No collectives.

Layout notes (hardware legality):
 - heads are padded 24 -> 32 partitions so all per-head matmul operands sit at
   32-aligned partition bases;
 - PE-transpose outputs must start at PSUM partition 0, so all transposes are
   emitted at base 0 and consumers slice 32-aligned blocks;
 - b1's padded bias rows carry 1.0 for k/v streams, which simultaneously gives
   the distance-constant fold and free per-cluster counts after transposition.
"""

import sys
import numpy as np

try:
    import concourse.bass as bass
except ImportError:  # pragma: no cover
    sys.path.insert(0, "/opt/trn_rl_repo")
    import concourse.bass as bass

from contextlib import ExitStack

import concourse.tile as tile
import concourse.mybir as mybir
from concourse.bass_utils import run_bass_kernel_spmd

f32 = mybir.dt.float32
f32r = mybir.dt.float32r
AX = mybir.AxisListType
ALU = mybir.AluOpType
ACTF = mybir.ActivationFunctionType

# problem dims
B, CIN, H, W = 16, 384, 64, 64
HEADS, HD, HP = 4, 24, 32          # head dim padded 24 -> 32
NSTREAM = 5
AG, CL = 8, 2
NPIX = AG * AG                      # 64 agent pixels
NCORES = 8
BPC = B // NCORES                   # batches per core = 2

_CACHE = {}


def _view(t, dims, offset_elems=0):
    """Strided free-dim view of a 2D tile: dims = [[step, count], ...]."""
    return bass.AP(tensor=t.tensor, offset=t.offset + offset_elems,
                   ap=[list(t.ap[0])] + [list(d) for d in dims])


def _upsample_R():
    # jax.image.resize(method='bilinear') 8 -> 64: triangle kernel, half-pixel
    # centers, weights normalized per output row. Verified exact vs jax.
    o = np.arange(64)
    t = (o + 0.5) * (8.0 / 64.0) - 0.5
    i = np.arange(8)
    w = np.maximum(0.0, 1.0 - np.abs(t[:, None] - i[None, :]))
    w = w / w.sum(axis=1, keepdims=True)
    return w.astype(np.float32)


def _split_multi_waits(nc):
    """This container's walrus rejects >1 semaphore wait per instruction;
    move extra waits onto same-engine no-ops inserted just before."""
    n = 0
    for fn in nc.m.functions:
        for bb in fn.blocks:
            new_list = []
            for inst in bb.instructions:
                si = inst.sync_info
                if si is not None and si.on_wait and len(si.on_wait) > 1:
                    waits = list(si.on_wait)
                    for wt in waits[:-1]:
                        nop = mybir.InstNoOp(
                            name=f"waitsplit-{n}", engine=inst.engine,
                            ins=[], outs=[],
                            sync_info=mybir.SyncInfo(on_wait=[wt], on_update=[]),
                        )
                        n += 1
                        new_list.append(nop)
                    si.on_wait = [waits[-1]]
                new_list.append(inst)
            bb.instructions = new_list
    return n


def _build_nc():
    nc = bass.Bass("TRN2")
    xc = nc.dram_tensor("xc", (BPC, CIN, H, W), f32, kind="ExternalInput")
    w1p = nc.dram_tensor("w1p", (128, 1920), f32, kind="ExternalInput")
    w2p = nc.dram_tensor("w2p", (128, 384), f32, kind="ExternalInput")
    b1p = nc.dram_tensor("b1p", (128, 5), f32, kind="ExternalInput")
    b2p = nc.dram_tensor("b2p", (128, 3), f32, kind="ExternalInput")
    upw = nc.dram_tensor("upw", (64, 4096), f32, kind="ExternalInput")
    i128 = nc.dram_tensor("i128", (128, 128), f32, kind="ExternalInput")
    e1 = nc.dram_tensor("e1", (128, 1), f32, kind="ExternalInput")
    e1h = nc.dram_tensor("e1h", (128, 4), f32, kind="ExternalInput")
    alphas = nc.dram_tensor("alphas", (4, 2), f32, kind="ExternalInput")
    betab = nc.dram_tensor("betab", (64, 32), f32, kind="ExternalInput")
    outc = nc.dram_tensor("outc", (BPC, CIN, H, W), f32, kind="ExternalOutput")

    def mm(out, lhsT, rhs, start=True, stop=True, tp=(0, 0)):
        nc.tensor.matmul(
            out, lhsT=lhsT, rhs=rhs, start=start, stop=stop,
            tile_position=tp,
        )

    with tile.TileContext(nc) as tc, ExitStack() as ctx:
        const = ctx.enter_context(tc.tile_pool(name="const", bufs=1))
        xin = ctx.enter_context(tc.tile_pool(name="xin", bufs=3))
        mid = ctx.enter_context(tc.tile_pool(name="mid", bufs=2))
        outp = ctx.enter_context(tc.tile_pool(name="outp", bufs=3))
        ps = ctx.enter_context(tc.tile_pool(name="ps", bufs=4, space="PSUM"))
        psU = ctx.enter_context(tc.tile_pool(name="psU", bufs=4, space="PSUM"))

        tW1 = const.tile([128, 1920], f32)
        nc.sync.dma_start(out=tW1, in_=w1p[:, :])
        tW2 = const.tile([128, 384], f32)
        nc.sync.dma_start(out=tW2, in_=w2p[:, :])
        tW2r = const.tile([128, 384], f32r)
        nc.vector.tensor_copy(tW2r, tW2)
        tB1 = const.tile([128, 5], f32)
        nc.sync.dma_start(out=tB1, in_=b1p[:, :])
        tB2 = const.tile([128, 3], f32)
        nc.sync.dma_start(out=tB2, in_=b2p[:, :])
        tUP = const.tile([64, 4096], f32)
        nc.sync.dma_start(out=tUP, in_=upw[:, :])
        tUPr = const.tile([64, 4096], f32r)
        nc.scalar.activation(out=tUPr, in_=tUP, func=ACTF.Copy)
        tI = const.tile([128, 128], f32)
        nc.sync.dma_start(out=tI, in_=i128[:, :])
        tE1 = const.tile([128, 1], f32)
        nc.sync.dma_start(out=tE1, in_=e1[:, :])
        tE1H = const.tile([128, 4], f32)
        nc.sync.dma_start(out=tE1H, in_=e1h[:, :])
        tE1Hr = const.tile([128, 4], f32r)
        nc.vector.tensor_copy(tE1Hr, tE1H)
        tAL = const.tile([4, 2], f32)
        nc.sync.dma_start(out=tAL, in_=alphas[:, :])
        tBB = const.tile([64, 32], f32)
        nc.sync.dma_start(out=tBB, in_=betab[:, :])
        tONES = const.tile([1, 64], f32)
        nc.vector.memset(tONES, 1.0)

        # ---- per-batch: load + pool -> project ----
        Ss = []
        for b in range(BPC):
            xp = mid.tile([128, 192], f32)
            for j in range(3):
                xt = xin.tile([128, 4096], f32)
                nc.sync.dma_start(
                    out=xt,
                    in_=xc[b, j * 128:(j + 1) * 128].rearrange("c h w -> c (h w)"))
                nc.vector.tensor_reduce(
                    out=xp[:, j * 64:(j + 1) * 64],
                    in_=xt.rearrange("p (hb hi wb wi) -> p hb wb hi wi",
                                     hb=8, hi=8, wb=8, wi=8),
                    axis=AX.XY, op=ALU.add)
            py = ps.tile([128, 320], f32, tag="ps")
            for s in range(5):
                for j in range(3):
                    mm(py[:, s * 64:(s + 1) * 64],
                       lhsT=tW1[:, j * 640 + s * 128: j * 640 + (s + 1) * 128],
                       rhs=xp[:, j * 64:(j + 1) * 64],
                       start=(j == 0), stop=(j == 2))
            S = mid.tile([128, 320], f32)
            for s in range(5):
                nc.scalar.activation(
                    out=S[:, s * 64:(s + 1) * 64], in_=py[:, s * 64:(s + 1) * 64],
                    func=ACTF.Identity, bias=tB1[:, s:s + 1], scale=1.0 / 64.0)
            Ss.append(S)

        for b in range(BPC):
            S = Ss[b]
        for b in range(BPC):
            S = Ss[b]
            # ---- cluster pools (sum of 16): KC cols (si, m), si=k0,v0,k1,v1 ----
            KC = mid.tile([128, 16], f32)
            for si in range(4):
                sc = (si + 1) * 64
                nc.vector.tensor_reduce(
                    out=KC[:, si * 4:(si + 1) * 4],
                    in_=S[:, sc:sc + 64].rearrange(
                        "p (mi ii mj jj) -> p mi mj ii jj", mi=2, ii=4, mj=2, jj=4),
                    axis=AX.XY, op=ALU.add)
            KC2 = mid.tile([128, 16], f32)
            nc.vector.tensor_mul(KC2, KC, KC)

            # ---- block-diagonal kc (per mod): KCBD[h*32+c, h*4+m] ----
            KCBD = [None, None]
            KC2BD = [None, None]
            for j in range(2):
                si = 2 * j
                kcbd = mid.tile([128, 16], f32)
                nc.vector.memset(kcbd, 0.0)
                for h in range(4):
                    nc.vector.tensor_copy(
                        kcbd[h * 32:(h + 1) * 32, h * 4:(h + 1) * 4],
                        KC[h * 32:(h + 1) * 32, si * 4:(si + 1) * 4])
                kc2bd = mid.tile([128, 16], f32)
                nc.vector.tensor_mul(kc2bd, kcbd, kcbd)
                KCBD[j] = kcbd
                KC2BD[j] = kc2bd

            # ---- cluster sq-norms row [1, 32] cols (j, h, m); E1 = 1/256 mask ----
            pn2 = ps.tile([1, 32], f32, tag="ps")
            for j in range(2):
                mm(pn2[0:1, j * 16:(j + 1) * 16], lhsT=tE1, rhs=KC2BD[j])
            n2row = mid.tile([1, 32], f32)
            nc.vector.tensor_copy(n2row, pn2)
            pn2b = ps.tile([64, 32], f32, tag="ps")
            mm(pn2b, lhsT=tONES, rhs=n2row)
            N2B = mid.tile([64, 32], f32)
            nc.vector.tensor_copy(N2B, pn2b)

            # ---- simT[n, (j,h,m)] = kc.k (x16, +16) via block-diag rhs ----
            SIM = mid.tile([64, 32], f32)
            for j in range(2):
                s = 1 + 2 * j
                psimt = ps.tile([64, 16], f32, tag="ps")
                mm(psimt, lhsT=S[:, s * 64:(s + 1) * 64], rhs=KCBD[j])
                # sim = 0.125*cross_raw - ||kc||^2 (= 2 kc.k - ||kc||^2 + const)
                nc.vector.scalar_tensor_tensor(
                    out=SIM[:, j * 16:(j + 1) * 16], in0=psimt, scalar=0.125,
                    in1=N2B[:, j * 16:(j + 1) * 16],
                    op0=ALU.mult, op1=ALU.subtract)

            # ---- assignment weights WT[n, (j,h,m)] ----
            WT = mid.tile([64, 32], f32r)
            HMAX = mid.tile([64, 4], f32)
            nc.vector.tensor_reduce(
                out=HMAX, in_=SIM[:, 0:16].rearrange("p (h m) -> p h m", h=4),
                axis=AX.X, op=ALU.max)
            nc.vector.tensor_tensor(
                out=WT[:, 0:16].rearrange("p (h m) -> p h m", h=4),
                in0=SIM[:, 0:16].rearrange("p (h m) -> p h m", h=4),
                in1=_view(HMAX, [[1, 4], [0, 4]]), op=ALU.is_ge)
            ESOFT = mid.tile([64, 16], f32)
            nc.scalar.activation(out=ESOFT, in_=SIM[:, 16:32], func=ACTF.Exp)
            SSUM = mid.tile([64, 4], f32)
            nc.vector.tensor_reduce(
                out=SSUM, in_=ESOFT.rearrange("p (h m) -> p h m", h=4),
                axis=AX.X, op=ALU.add)
            SRCP = mid.tile([64, 4], f32)
            nc.vector.reciprocal(SRCP, SSUM)
            nc.vector.tensor_tensor(
                out=WT[:, 16:32].rearrange("p (h m) -> p h m", h=4),
                in0=ESOFT.rearrange("p (h m) -> p h m", h=4),
                in1=_view(SRCP, [[1, 4], [0, 4]]), op=ALU.mult)

            # ---- per-mod agg in [4(m), 128(h,c pad)] layout ----
            PAGGNT = ps.tile([128, 8], f32, tag="ps")
            PAGGT = ps.tile([128, 8], f32, tag="ps")
            for j in range(2):
                s = 2 + 2 * j
                si = 1 + 2 * j
                pstv = ps.tile([64, 128], f32, tag="ps")
                nc.tensor.transpose(out=pstv, in_=S[:, s * 64:(s + 1) * 64],
                                    identity=tI)
                stv = mid.tile([64, 128], f32r)
                nc.vector.tensor_copy(stv, pstv)

                pagg = ps.tile([4, 104], f32, tag="ps")
                for h in range(4):
                    mm(pagg[0:4, h * 26:(h + 1) * 26],
                       lhsT=WT[0:64, j * 16 + h * 4: j * 16 + (h + 1) * 4],
                       rhs=stv[0:64, h * 32: h * 32 + 26])
                pvct = ps.tile([4, 128], f32, tag="ps")
                nc.tensor.transpose(out=pvct, in_=KC[:, si * 4:(si + 1) * 4],
                                    identity=tI)
                vcts = mid.tile([4, 128], f32)
                nc.vector.tensor_copy(vcts, pvct)

                rc = mid.tile([4, 4], f32)
                nc.vector.tensor_scalar_add(rc, _view(pagg, [[26, 4]], 24), 1.0)
                nc.vector.reciprocal(rc, rc)
                agg = mid.tile([4, 128], f32)
                nc.vector.memset(agg, 0.0)
                # agg = (vc/16 + sum_n w v) * 1/(1+count)
                nc.vector.scalar_tensor_tensor(
                    out=_view(agg, [[32, 4], [1, 24]]),
                    in0=_view(vcts, [[32, 4], [1, 24]]), scalar=1.0 / 16.0,
                    in1=_view(pagg, [[26, 4], [1, 24]]),
                    op0=ALU.mult, op1=ALU.add)
                nc.vector.tensor_tensor(
                    out=_view(agg, [[32, 4], [1, 24]]),
                    in0=_view(agg, [[32, 4], [1, 24]]),
                    in1=_view(rc, [[1, 4], [0, 24]]), op=ALU.mult)

                # normalized + alpha-scaled aggn
                sq = mid.tile([4, 128], f32)
                nc.vector.tensor_mul(sq, agg, agg)
                ns = mid.tile([4, 4], f32)
                nc.vector.tensor_reduce(
                    out=ns, in_=_view(sq, [[32, 4], [1, 24]]),
                    axis=AX.X, op=ALU.add)
                lnv = mid.tile([4, 4], f32)
                nc.scalar.activation(lnv, ns, func=ACTF.Ln)
                sd = mid.tile([4, 4], f32)
                nc.scalar.activation(sd, lnv, func=ACTF.Exp, scale=0.5)  # sqrt
                nc.vector.tensor_scalar_add(sd, sd, 1e-6)
                rn = mid.tile([4, 4], f32)
                nc.vector.reciprocal(rn, sd)
                aggn = mid.tile([4, 128], f32)
                nc.vector.memset(aggn, 0.0)
                nc.vector.scalar_tensor_tensor(
                    out=_view(aggn, [[32, 4], [1, 24]]),
                    in0=_view(agg, [[32, 4], [1, 24]]), scalar=tAL[:, j:j + 1],
                    in1=_view(rn, [[1, 4], [0, 24]]),
                    op0=ALU.mult, op1=ALU.mult)

                nc.tensor.transpose(out=PAGGNT[:, j * 4:(j + 1) * 4], in_=aggn,
                                    identity=tI[0:4, 0:4])
                nc.tensor.transpose(out=PAGGT[:, j * 4:(j + 1) * 4], in_=agg,
                                    identity=tI[0:4, 0:4])

            # block-diagonal agg / aggn: [128(h*32+c), 32(h*8+j*4+m)]
            AGGNBD = mid.tile([128, 32], f32r)
            nc.vector.memset(AGGNBD.bitcast(f32), 0.0)
            AGGBD = mid.tile([128, 32], f32r)
            nc.vector.memset(AGGBD.bitcast(f32), 0.0)
            for h in range(4):
                nc.vector.tensor_copy(
                    AGGNBD[h * 32:(h + 1) * 32, h * 8:(h + 1) * 8],
                    PAGGNT[h * 32:(h + 1) * 32, 0:8])
                nc.vector.tensor_copy(
                    AGGBD[h * 32:(h + 1) * 32, h * 8:(h + 1) * 8],
                    PAGGT[h * 32:(h + 1) * 32, 0:8])

            # ---- pa reciprocal norms [64, 4] per head ----
            s0sq = mid.tile([128, 64], f32r)
            nc.vector.tensor_mul(s0sq, S[:, 0:64], S[:, 0:64])
            ppn2 = ps.tile([64, 4], f32, tag="ps")
            mm(ppn2, lhsT=s0sq, rhs=tE1Hr)
            pad_ = mid.tile([64, 4], f32)
            nc.scalar.activation(pad_, ppn2, func=ACTF.Ln, scale=256.0)
            par = mid.tile([64, 4], f32)
            nc.scalar.activation(par, pad_, func=ACTF.Exp, scale=0.5)  # ||pa||
            nc.vector.tensor_scalar_add(par, par, 1e-6)
            nc.vector.reciprocal(par, par)

            # ---- cosine sim + assignment softmax over all 8 clusters ----
            s0r = mid.tile([128, 64], f32r)
            nc.vector.tensor_copy(s0r, S[:, 0:64])
            psimcos = ps.tile([64, 32], f32, tag="ps")
            mm(psimcos, lhsT=s0r, rhs=AGGNBD)
            SC = mid.tile([64, 32], f32)
            nc.vector.tensor_tensor(
                out=SC, in0=psimcos, in1=_view(par, [[1, 4], [0, 8]]),
                op=ALU.mult)
            nc.vector.tensor_add(SC, SC, tBB)
            EA = mid.tile([64, 32], f32)
            nc.scalar.activation(EA, SC, func=ACTF.Exp)
            ASUM = mid.tile([64, 4], f32)
            nc.vector.tensor_reduce(
                out=ASUM, in_=EA.rearrange("p (h m) -> p h m", h=4),
                axis=AX.X, op=ALU.add)
            ARCP = mid.tile([64, 4], f32)
            nc.vector.reciprocal(ARCP, ASUM)
            ASSC = mid.tile([64, 32], f32)   # assign, cols (h, j, m)
            nc.vector.tensor_tensor(
                out=ASSC, in0=EA,
                in1=_view(ARCP, [[1, 4], [0, 8]]), op=ALU.mult)
            past = ps.tile([32, 64], f32, tag="ps")
            nc.tensor.transpose(out=past, in_=ASSC, identity=tI[0:64, 0:64])
            asts = mid.tile([32, 64], f32r)
            nc.vector.tensor_copy(asts, past)

            # ---- G = agg @ W2: [32 (h,j,m), 384], then q8T = asts.T @ G ----
            pg32 = ps.tile([32, 384], f32, tag="ps")
            mm(pg32, lhsT=AGGBD, rhs=tW2r)
            gs = mid.tile([32, 384], f32r)
            nc.scalar.copy(gs, pg32)

            pq8t = ps.tile([64, 384], f32, tag="ps")
            mm(pq8t, lhsT=asts, rhs=gs)
            q8t = mid.tile([64, 384], f32r)
            nc.vector.tensor_copy(q8t, pq8t)

            # ---- bilinear upsample 8x8 -> 64x64 via matmul; +b2; DMA out ----
            for jo in range(3):
                osb = outp.tile([128, 4096], f32)
                for nn in range(8):
                    pup = psU.tile([128, 512], f32, tag="psU")
                    mm(pup, lhsT=q8t[0:64, jo * 128:(jo + 1) * 128],
                       rhs=tUPr[0:64, nn * 512:(nn + 1) * 512])
                    if nn % 2 == 0:
                        nc.scalar.activation(
                            out=osb[:, nn * 512:(nn + 1) * 512], in_=pup,
                            func=ACTF.Identity, bias=tB2[:, jo:jo + 1], scale=1.0)
                    else:
                        nc.vector.tensor_scalar_add(
                            osb[:, nn * 512:(nn + 1) * 512], pup,
                            tB2[:, jo:jo + 1])
                oj = outc[b, jo * 128:(jo + 1) * 128].rearrange("c h w -> c (h w)")
                nc.sync.dma_start(out=oj[:, 0:2048], in_=osb[:, 0:2048])
                nc.sync.dma_start(out=oj[:, 2048:4096], in_=osb[:, 2048:4096])

    _split_multi_waits(nc)
    return nc


def _host_prep(W1, b1, W2, b2, sim_alpha, sim_beta):
    W1 = np.asarray(W1, np.float32)
    b1 = np.asarray(b1, np.float32)
    W2 = np.asarray(W2, np.float32)
    b2 = np.asarray(b2, np.float32)
    sim_alpha = np.asarray(sim_alpha, np.float32)
    sim_beta = np.asarray(sim_beta, np.float32)

    # W1 padded: [3, 128, 5 streams, 4 heads, 32] -> (128, 1920)
    w1r = W1.reshape(3, 128, NSTREAM, HEADS, HD)
    w1p = np.zeros((3, 128, NSTREAM, HEADS, HP), np.float32)
    w1p[..., :HD] = w1r
    w1p = w1p.reshape(3, 128, NSTREAM * 128).transpose(1, 0, 2).reshape(128, 1920)

    # b1 padded: (128, 5); pad row 24 carries 1.0 for k/v streams
    b1r = b1.reshape(NSTREAM, HEADS, HD)
    b1pad = np.zeros((NSTREAM, HEADS, HP), np.float32)
    b1pad[..., :HD] = b1r
    for s in range(1, NSTREAM):
        b1pad[s, :, HD] = 1.0
    b1p = b1pad.transpose(1, 2, 0).reshape(128, NSTREAM)

    # W2 padded rows 24->32: (128, 384)
    w2r = W2.reshape(HEADS, HD, CIN)
    w2p = np.zeros((HEADS, HP, CIN), np.float32)
    w2p[:, :HD] = w2r
    w2p = w2p.reshape(128, CIN)

    b2p = b2.reshape(3, 128).T.copy()

    R = _upsample_R()
    A = R.T  # (8 in, 64 out)
    upw = (A[:, None, :, None] * A[None, :, None, :]).reshape(64, 4096)
    upw = np.ascontiguousarray(upw, np.float32)

    i128 = np.eye(128, dtype=np.float32)
    e1 = ((np.arange(128) % HP) < HD).astype(np.float32)[:, None] / 256.0
    e1h = np.zeros((128, 4), np.float32)
    for h2 in range(4):
        e1h[h2 * HP: h2 * HP + HD, h2] = 1.0 / 256.0

    alphas = np.empty((4, 2), np.float32)
    for j in range(2):
        alphas[:, j] = sim_alpha[j * 4:(j + 1) * 4]
    betab = np.empty((64, 32), np.float32)
    for h2 in range(4):
        for j in range(2):
            betab[:, h2 * 8 + j * 4: h2 * 8 + (j + 1) * 4] = sim_beta[
                j * 4:(j + 1) * 4][None, :]

    return dict(w1p=w1p, w2p=w2p, b1p=b1p, b2p=b2p, upw=upw, i128=i128,
                e1=e1.astype(np.float32), e1h=e1h, alphas=alphas, betab=betab)


def _get_nc():
    if "nc" not in _CACHE:
        _CACHE["nc"] = _build_nc()
    return _CACHE["nc"]


def run(inputs, trace=False):
    nc = _get_nc()
    consts = _host_prep(inputs["W1"], inputs["b1"], inputs["W2"], inputs["b2"],
                        inputs["sim_alpha"], inputs["sim_beta"])
    x = np.ascontiguousarray(np.asarray(inputs["x"], np.float32))
    in_maps = []
    for i in range(NCORES):
        m = {"xc": np.ascontiguousarray(x[i * BPC:(i + 1) * BPC])}
        m.update(consts)
        in_maps.append(m)
    res = run_bass_kernel_spmd(nc, in_maps, core_ids=list(range(NCORES)),
                               trace=trace)
    out = np.concatenate([res.results[i]["outc"] for i in range(NCORES)], axis=0)
    return out, res


def kernel(**inputs):
    out, _ = run(inputs, trace=False)
    return out

